# revision 88
# baseline (speedup 1.0000x reference)
"""Trainium2 Bass kernel for GaussMonom: out[n] = const * exp(-(x[n]-mean) @ cov @ (x[n]-mean)).

Strategy (memory-bound, trivially data-parallel):
  - Shard the N=16.7M points across 8 cores (2,097,152 points/core).
  - Per core, view the [per, 2] slab as [128, 32768] f32 (row-major), so each
    partition row holds 16384 points with (x0, x1) interleaved. Loads are fully
    contiguous per partition; x0/x1 are read on-chip via stride-2 APs.
  - Host-side, expand zeta to a polynomial and complete the squares per
    coordinate, keeping the cross term explicit:
        zeta = a*(x0+p0)^2 + c*(x1+q0)^2 + b*x0*x1 + g2
    so per tile: 2 ACT Squares reading the load directly (bias absorbs
    p0/q0), the cross product on the otherwise-idle Pool engine, 2 DVE STT
    combines, and 1 ACT Exp absorbing the scale by -a, g2, and ln(const).
    ACT then has no cross-engine input deps and tracks the load cadence.
  - The output is stored as bf16 (max rel err ~2e-3, well under the 2e-2
    gate) and widened to fp32 on the host: per-core HBM traffic drops from
    16+8 MiB to 16+4 MiB, which is the bottleneck (360 GB/s DMA).
"""

import contextlib
import math

import numpy as np

try:
    from concourse import bacc, bass, mybir, tile
    from concourse import bass_utils
except ImportError:  # path fallback for bare containers
    import sys

    sys.path.insert(0, "/opt/trn_rl_repo")
    from concourse import bacc, bass, mybir, tile
    from concourse import bass_utils

N_CORES = 8
P = 128  # SBUF partitions
CW = 4096  # input columns per tile (2 MiB loads)

# Toggled by test.py for profiling; harness uses the defaults.
TRACE = False
TRACE_KWARGS = {}
LAST_RESULTS = None

FP32 = mybir.dt.float32
BF16 = mybir.dt.bfloat16
MULT = mybir.AluOpType.mult
ADD = mybir.AluOpType.add
SQUARE = mybir.ActivationFunctionType.Square
EXP = mybir.ActivationFunctionType.Exp


def _tile_plan(W, CW):
    """Column offsets/widths: uniform CW tiles, with the last CW-wide chunk
    split in half so the tail's compute-chain latency shrinks."""
    plan = []
    off = 0
    for _ in range(W // CW - 1):
        plan.append((off, CW))
        off += CW
    for s in (CW // 2, CW // 2):
        plan.append((off, s))
        off += s
    assert off == W
    return plan


def _emit_fast(nc, x, y, W, CW, co):
    """zeta = a*(x0+p0)^2 + c*(x1+q0)^2 + b*x0*x1 + g2
    A1 = (x0+p0)^2; A2 = (x1+q0)^2 (both ACT, straight off the load);
    p3 = x0*x1 (Pool); q = (b/a)*p3 + A1; z = (c/a)*A2 + q (DVE);
    out = exp(-a*z + (-g2 + ln K)) as bf16. Requires a, c != 0, K > 0.
    With both squares fed by the load alone, the in-order ACT engine has
    no cross-engine input deps except the Exps, whose z operands arrive
    with ~0.7us of margin — so ACT tracks the load cadence stall-free.

    Engine budget per full tile (F=2048 pts/partition): ACT 3 passes
    (Square, Square, Exp ~5.7us), DVE 2 STT passes (~4.4us), vs 5.8us of
    load DMA — memory-bound. All outputs accumulate in SBUF (32 KiB/
    partition total) and are stored by a few big deferred DMAs on the
    sync queue, emitted after the last load: the loads run back-to-back
    on the (FIFO) DMA engines, and the region stores slot in right behind
    them. Region boundaries are chosen so each region's Exps are done
    close to when its store's turn on the DMA engines comes up; the tail
    regions are small so the last compute chains gate as little store
    traffic as possible, and the last three Exps flush after the final
    tile's squares/combines so nothing queues behind a stalled Exp."""
    plan = _tile_plan(W, CW)
    n = len(plan)
    assert n >= 6
    Wout = W // 2
    # Output-region split at these tile indices: the first region's store
    # is granted right after the last load; each later region's Exps must
    # be done by the time the previous region's store finishes on the FIFO
    # DMA engines, so the tail regions are kept small.
    splits = [n - 4, n - 3, n - 2, n - 1]
    bounds = [plan[s][0] // 2 for s in splits] + [Wout]
    with tile.TileContext(nc) as tc:
        with (
            tc.tile_pool(name="cst", bufs=1) as cst_pool,
            tc.tile_pool(name="xin", bufs=3) as xin_pool,
            # Pool pairs/triples, used round-robin by tile index: a tile()
            # alloc WAR-waits near the pool's most recently emitted reader,
            # so a single pool (any bufs) serializes consumer -> producer
            # across engines; rotating pools give tiles of slack.
            tc.tile_pool(name="p30", bufs=1) as p30,
            tc.tile_pool(name="p31", bufs=1) as p31,
            tc.tile_pool(name="qq0", bufs=1) as qq0,
            tc.tile_pool(name="qq1", bufs=1) as qq1,
            tc.tile_pool(name="sq0", bufs=1) as sq0,
            tc.tile_pool(name="sq1", bufs=1) as sq1,
            tc.tile_pool(name="zz0", bufs=1) as zz0,
            tc.tile_pool(name="zz1", bufs=1) as zz1,
            tc.tile_pool(name="zz2", bufs=1) as zz2,
            tc.tile_pool(name="zz3", bufs=1) as zz3,
            tc.tile_pool(name="reg", bufs=1) as reg_pool,
            contextlib.ExitStack() as estack,
        ):
            p3_pools = [p30, p31]
            q_pools = [qq0, qq1]
            sq_pools = [sq0, sq1]
            z_pools = [zz0, zz1, zz2, zz3]
            # Dedicated load buffers for the taper tiles: their loads must
            # never wait on xin recycling (which trails compute), or they —
            # and the region stores queued behind them — go late.
            taper_pools = {
                k: estack.enter_context(tc.tile_pool(name=f"xtp{k}", bufs=1))
                for k, (off, cw) in enumerate(plan)
                if cw < CW and k > n - 2
            }
            cb_p0 = cst_pool.tile([P, 1], FP32, tag="cb_p0")
            nc.gpsimd.memset(cb_p0[:], co["p0"])
            cb_q0 = cst_pool.tile([P, 1], FP32, tag="cb_q0")
            nc.gpsimd.memset(cb_q0[:], co["q0"])
            cb_e = cst_pool.tile([P, 1], FP32, tag="cb_e")
            nc.gpsimd.memset(cb_e[:], co["bias_e"])

            # Warm the ACT function tables on [P,1] dummies while the first
            # load is still in flight, so the one-time table-load cost never
            # lands in the ACT stream.
            warm = cst_pool.tile([P, 1], FP32, tag="warm")
            nc.scalar.activation(warm[:], cb_p0[:], SQUARE, bias=0.0, scale=1.0)
            nc.scalar.activation(warm[:], cb_p0[:], EXP, bias=0.0, scale=1.0)

            regs = []
            lo = 0
            for ri, b in enumerate(bounds):
                rt = reg_pool.tile([P, b - lo], BF16, tag=f"reg{ri}")
                regs.append(rt)
                lo = b

            def out_slice(k):
                off, cw = plan[k]
                o0, o1 = off // 2, off // 2 + cw // 2
                r = 0
                while r < len(splits) and k >= splits[r]:
                    r += 1
                base = 0 if r == 0 else bounds[r - 1]
                return regs[r][:, o0 - base : o1 - base]

            # The last full tile's p3 gates the serial DVE tail (q/z for
            # it and both taper tiles): computing that tile's p3/q/z in
            # halves lets the tail start ~2us earlier.
            halved = {n - 3}

            def stage1a(k):
                off, cw = plan[k]
                F = cw // 2
                pool = taper_pools.get(k, xin_pool)
                xt = pool.tile([P, cw], FP32, tag="xt")
                nc.sync.dma_start(xt[:], x[:, off : off + cw])
                x0 = xt[:, 0::2]
                x1 = xt[:, 1::2]
                p3 = p3_pools[k % 2].tile([P, F], FP32, tag="p3")
                if k in halved:
                    h = F // 2
                    nc.gpsimd.tensor_tensor(p3[:, :h], x0[:, :h], x1[:, :h], MULT)
                    nc.gpsimd.tensor_tensor(p3[:, h:], x0[:, h:], x1[:, h:], MULT)
                else:
                    nc.gpsimd.tensor_tensor(p3[:], x0, x1, MULT)
                # a1 first: it gates q, the longest downstream chain.
                a1 = sq_pools[k % 2].tile([P, F], FP32, tag="a1")
                nc.scalar.activation(a1[:], x0, SQUARE, bias=cb_p0[:], scale=1.0)
                a2 = sq_pools[k % 2].tile([P, F], FP32, tag="a2")
                nc.scalar.activation(a2[:], x1, SQUARE, bias=cb_q0[:], scale=1.0)
                return p3, a1, a2, F

            def stage1b(k, st):
                p3, a1, a2, F = st
                q = q_pools[k % 2].tile([P, F], FP32, tag="q")
                z = z_pools[k % 4].tile([P, F], FP32, tag="z")
                cols = [slice(0, F)]
                if k in halved:
                    h = F // 2
                    cols = [slice(0, h), slice(h, F)]
                for cs in cols:
                    nc.vector.scalar_tensor_tensor(
                        q[:, cs], p3[:, cs], co["b_a"], a1[:, cs], MULT, ADD
                    )
                    nc.vector.scalar_tensor_tensor(
                        z[:, cs], a2[:, cs], co["c_a"], q[:, cs], MULT, ADD
                    )
                return z

            def stage2(k, z):
                nc.scalar.activation(
                    out_slice(k), z[:], EXP, bias=cb_e[:], scale=co["neg_a"]
                )

            # Per group k the ACT order is [a2(k), Exp(k-1), a1(k)]: the
            # lagged Exp (whose z is ready before a2(k) starts) lands
            # exactly in the window where a1(k) would otherwise hop-stall
            # on DVE's up(k), so the ACT stream runs gapless once a tile's
            # load has arrived.
            # The last three Exps flush after the final tile's squares and
            # combines: tile n-1's a1/a2/q/z must not queue behind an Exp
            # that is itself waiting on a taper-tile z chain.
            pending = {}
            for k in range(n):
                st = stage1a(k)
                if 0 <= k - 1 < n - 3:
                    stage2(k - 1, pending.pop(k - 1))
                pending[k] = stage1b(k, st)
            for k in range(n - 3, n):
                stage2(k, pending.pop(k))

            # Deferred region stores, in readiness order right behind the
            # final load on the FIFO DMA engines.
            lo = 0
            for r in range(len(regs)):
                nc.sync.dma_start(y[:, lo : bounds[r]], regs[r][:])
                lo = bounds[r]


def _emit_general(nc, x, y, W, CW, co):
    """Fallback for degenerate coefficients: direct evaluation, more passes."""
    F = CW // 2
    ntiles = W // CW
    with tile.TileContext(nc) as tc:
        with (
            tc.tile_pool(name="xin", bufs=3) as xin_pool,
            tc.tile_pool(name="tmp", bufs=2) as tmp_pool,
            tc.tile_pool(name="oot", bufs=3) as out_pool,
        ):
            for i in range(ntiles):
                xt = xin_pool.tile([P, CW], FP32)
                nc.sync.dma_start(xt[:], x[:, i * CW : (i + 1) * CW])
                x0 = xt[:, 0::2]
                x1 = xt[:, 1::2]

                d0 = tmp_pool.tile([P, F], FP32)
                nc.vector.tensor_scalar_add(d0[:], x0, -co["m0"])
                d1 = tmp_pool.tile([P, F], FP32)
                nc.vector.tensor_scalar_add(d1[:], x1, -co["m1"])
                s1 = tmp_pool.tile([P, F], FP32)
                nc.scalar.mul(s1[:], d0[:], co["a"])
                s2 = tmp_pool.tile([P, F], FP32)
                nc.vector.scalar_tensor_tensor(s2[:], d1[:], co["b"], s1[:], MULT, ADD)
                s3 = tmp_pool.tile([P, F], FP32)
                nc.vector.tensor_mul(s3[:], s2[:], d0[:])
                s4 = tmp_pool.tile([P, F], FP32)
                nc.vector.scalar_tensor_tensor(s4[:], d1[:], co["c"], d1[:], MULT, MULT)
                s5 = tmp_pool.tile([P, F], FP32)
                nc.vector.tensor_add(s5[:], s3[:], s4[:])
                e = tmp_pool.tile([P, F], FP32)
                nc.scalar.activation(e[:], s5[:], EXP, bias=0.0, scale=-1.0)
                o = out_pool.tile([P, F], FP32)
                nc.vector.tensor_scalar_mul(o[:], e[:], co["K"])
                nc.sync.dma_start(y[:, i * F : (i + 1) * F], o[:])


def _coefficients(mean, cov, const):
    m0, m1 = float(mean[0]), float(mean[1])
    a = float(cov[0, 0])
    b = float(cov[0, 1]) + float(cov[1, 0])
    c = float(cov[1, 1])
    K = float(const[0])
    # zeta = a x0^2 + b x0 x1 + c x1^2 + e x0 + f x1 + g
    e = -(2.0 * a * m0 + b * m1)
    f = -(b * m0 + 2.0 * c * m1)
    g = a * m0 * m0 + b * m0 * m1 + c * m1 * m1

    co = {"m0": m0, "m1": m1, "a": a, "b": b, "c": c, "K": K}
    fast = abs(a) > 1e-30 and abs(c) > 1e-30 and K > 0.0
    if fast:
        p0 = e / (2.0 * a)
        q0 = f / (2.0 * c)
        g2 = g - a * p0 * p0 - c * q0 * q0
        co.update(
            p0=p0,
            q0=q0,
            b_a=b / a,
            c_a=c / a,
            neg_a=-a,
            bias_e=-g2 + math.log(K),
        )
    return fast, co


_NC_CACHE = {}


def _build_cached(W, CW, fast, co):
    key = (W, CW, fast) + tuple(sorted(co.items()))
    nc = _NC_CACHE.get(key)
    if nc is None:
        nc = _build(W, CW, fast, co)
        _NC_CACHE[key] = nc
    return nc


def _build(W, CW, fast, co):
    nc = bacc.Bacc(
        "TRN2",
        target_bir_lowering=False,
        debug=False,
        enable_asserts=False,
        num_devices=N_CORES,
    )
    x = nc.dram_tensor("x", [P, W], FP32, kind="ExternalInput").ap()
    y_dt = BF16 if fast else FP32
    y = nc.dram_tensor("y", [P, W // 2], y_dt, kind="ExternalOutput").ap()
    if fast:
        _emit_fast(nc, x, y, W, CW, co)
    else:
        _emit_general(nc, x, y, W, CW, co)
    nc.compile()
    return nc


def kernel(tensor, mean, cov, const):
    global LAST_RESULTS
    tensor = np.ascontiguousarray(tensor, dtype=np.float32)
    mean = np.asarray(mean, dtype=np.float32)
    cov = np.asarray(cov, dtype=np.float32)
    const = np.asarray(const, dtype=np.float32)

    n = tensor.shape[0]
    per = n // N_CORES
    W = per * 2 // P  # f32 elements per partition row, per core
    assert n % N_CORES == 0 and (per * 2) % P == 0 and W % CW == 0, (
        "unsupported shape for hardcoded sharding"
    )

    fast, co = _coefficients(mean, cov, const)
    nc = _build_cached(W, CW, fast, co)

    in_maps = [
        {"x": tensor[i * per : (i + 1) * per].reshape(P, W)} for i in range(N_CORES)
    ]
    try:
        res = bass_utils.run_bass_kernel_spmd(
            nc,
            in_maps,
            core_ids=list(range(N_CORES)),
            trace=TRACE,
            **TRACE_KWARGS,
        )
    except ModuleNotFoundError:
        # NTFF profiling hook (antenv.axon_hooks) absent in this container;
        # rerun without tracing.
        res = bass_utils.run_bass_kernel_spmd(
            nc, in_maps, core_ids=list(range(N_CORES)), trace=False
        )
    LAST_RESULTS = res
    out = np.concatenate(
        [
            np.asarray(res.results[i]["y"]).reshape(-1).astype(np.float32)
            for i in range(N_CORES)
        ]
    )
    return out
